# revision 12
# baseline (speedup 1.0000x reference)
"""Trainium2 Bass kernel for nn_CMVNet (moe_routing).

Reference computation:
    h = relu(x @ W1.T + b1)            # [N, HID]
    e = c[num]                         # [N] per-sample expert index
    y = einsum('noh,nh->no', We[e], h) + be[e]   # OUT=1
    out = sigmoid(y)                   # [N, 1]

Data-parallel over 8 cores (N/8 = 16384 rows each). Two device programs:

v2 (default): host sorts each shard's tokens by expert. Each 512-token
  block of the sorted order spans only a handful of distinct experts
  (<= 32 slots). Stage B then computes, for 4 blocks CONCURRENTLY (one
  per 32-column strip of the PE array via tile_position), the scores of
  each block's tokens against just that block's expert slots:
      stage A: hT[hid, n] = relu(W1 @ xT_sorted + b1)   (16 mm / chunk)
      stage B: ps[32j+s, t] = WeB[block 4g+j, slot s] . hT[:, t]
               4 strips x 8 hid-chunks of accumulating matmuls
      select:  masked = (ps + beB) * ohB; y4 = I4.T @ masked  ([4, 512])
      sigmoid on ACT, DMA out, host un-permutes.
v1 (fallback, used if any block spans > 32 experts): dense scores against
  all 100 experts + one-hot select.

Matmuls run in float32r (TF32: fp32 layout, 10-bit mantissa, full PE
rate). Host pre-rounds matmul inputs to TF32; on-chip producers
(relu / mask ops) write float32r directly.
"""

import numpy as np

N, D_IN, HID, OUT, E = 131072, 256, 1024, 1, 100
NCORES = 8
NSH = N // NCORES          # 16384 rows per core
EP = 128                   # experts padded to full partition dim (v1)
CHUNK = 512
D_T = D_IN // 128          # 2 contraction tiles for stage A
H_T = HID // 128           # 8 hid tiles
NBLK = NSH // CHUNK        # 32 sorted-token blocks per core (v2)
STRIPS = 4                 # concurrent col-strips on the PE array (v2)
SLOTS = 32                 # on-chip expert slots per block (v2)
SLOTS_DMA = 8              # slots actually stored/DMAd per block (v2)
GROUPS = NBLK // STRIPS    # 8 strip-groups per core (v2)

TRACE = False              # set by test harness for profiled runs
LAST_RESULTS = None        # BassKernelResults of the last run (for test.py)

_BUILT = {}                # (version, nsh) -> compiled Bass module


def _bf16_dt():
    import ml_dtypes
    return ml_dtypes.bfloat16


def _tf32_round(a):
    """Round fp32 ndarray to TF32 (10-bit mantissa), round-to-nearest-even."""
    u = np.ascontiguousarray(a, dtype=np.float32).view(np.uint32)
    u = (u + 0x00000FFF + ((u >> 13) & 1)) & np.uint32(0xFFFFE000)
    return u.view(np.float32)


def _mk_bass(nsh):
    from concourse import bacc
    return bacc.Bacc("TRN2", target_bir_lowering=False, debug=False)


# --------------------------------------------------------------------------
# v1: dense scores against all experts + one-hot select
# --------------------------------------------------------------------------
def _build_nc_v1(nsh):
    from contextlib import ExitStack

    import concourse.mybir as mybir
    import concourse.tile as tile

    fp32 = mybir.dt.float32
    fr = mybir.dt.float32r
    AF = mybir.ActivationFunctionType
    OP = mybir.AluOpType

    nchunk = nsh // CHUNK
    nc = _mk_bass(nsh)

    xT = nc.dram_tensor("xT", [D_IN, nsh], fr, kind="ExternalInput")
    w1T = nc.dram_tensor("w1T", [D_IN, HID], fr, kind="ExternalInput")
    b1c = nc.dram_tensor("b1c", [128, H_T], fp32, kind="ExternalInput")
    weT = nc.dram_tensor("weT", [HID, EP], fr, kind="ExternalInput")
    bec = nc.dram_tensor("bec", [EP, 1], fp32, kind="ExternalInput")
    oh = nc.dram_tensor("oh", [EP, nsh], fp32, kind="ExternalInput")
    y = nc.dram_tensor("y", [1, nsh], fp32, kind="ExternalOutput")

    xT_v = xT.rearrange("(c p) n -> p c n", p=128)
    w1T_v = w1T.rearrange("(c p) h -> p c h", p=128)
    weT_v = weT.rearrange("(k p) e -> p k e", p=128)

    with tile.TileContext(nc) as tc, ExitStack() as ctx:
        cpool = ctx.enter_context(tc.tile_pool(name="consts", bufs=1))
        xin = ctx.enter_context(tc.tile_pool(name="xin", bufs=3))
        ohin = ctx.enter_context(tc.tile_pool(name="ohin", bufs=3))
        hpool = ctx.enter_context(tc.tile_pool(name="h", bufs=2))
        mpool = ctx.enter_context(tc.tile_pool(name="masked", bufs=3))
        ypool = ctx.enter_context(tc.tile_pool(name="yrow", bufs=3))
        php = ctx.enter_context(tc.tile_pool(name="ph", bufs=3, space="PSUM"))
        psp = ctx.enter_context(tc.tile_pool(name="ps", bufs=2, space="PSUM"))
        pyp = ctx.enter_context(tc.tile_pool(name="py", bufs=2, space="PSUM"))

        w1_sb = cpool.tile([128, D_T, HID], fr)
        we_sb = cpool.tile([128, H_T, EP], fr)
        b1_sb = cpool.tile([128, H_T], fp32)
        be_sb = cpool.tile([128, 1], fp32)
        ones_f32 = cpool.tile([128, 1], fp32)
        ones_sb = cpool.tile([128, 1], fr)
        nc.sync.dma_start(w1_sb[:], w1T_v[:])
        nc.sync.dma_start(we_sb[:], weT_v[:])
        nc.sync.dma_start(b1_sb[:], b1c[:])
        nc.sync.dma_start(be_sb[:], bec[:])
        nc.vector.memset(ones_f32[:], 1.0)
        nc.vector.tensor_copy(ones_sb[:], ones_f32[:])

        for ci in range(nchunk):
            n0 = ci * CHUNK
            xts = xin.tile([128, D_T, CHUNK], fr)
            nc.sync.dma_start(xts[:], xT_v[:, :, n0:n0 + CHUNK])
            oh_sb = ohin.tile([128, CHUNK], fp32)
            nc.sync.dma_start(oh_sb[:], oh[:, n0:n0 + CHUNK])

            hT = hpool.tile([128, H_T, CHUNK], fr)
            phs = []
            for j in range(H_T):
                ph = php.tile([128, CHUNK], fp32)
                for c in range(D_T):
                    nc.tensor.matmul(
                        ph[:],
                        w1_sb[:, c, 128 * j:128 * (j + 1)],
                        xts[:, c, :],
                        start=(c == 0),
                        stop=(c == D_T - 1),
                    )
                phs.append(ph)
            for j in range(H_T):
                if j % 2 == 0:
                    nc.scalar.activation(
                        hT[:, j, :], phs[j][:], AF.Relu,
                        bias=b1_sb[:, j:j + 1], scale=1.0,
                    )
                else:
                    nc.vector.tensor_scalar(
                        hT[:, j, :], phs[j][:],
                        b1_sb[:, j:j + 1], 0.0,
                        OP.add, OP.max,
                    )

            ps = psp.tile([128, CHUNK], fp32)
            for j in range(H_T):
                nc.tensor.matmul(
                    ps[:],
                    we_sb[:, j, :],
                    hT[:, j, :],
                    start=(j == 0),
                    stop=(j == H_T - 1),
                )

            masked = mpool.tile([128, CHUNK], fr)
            nc.vector.scalar_tensor_tensor(
                masked[:], ps[:], be_sb[:, 0:1], oh_sb[:],
                OP.add, OP.mult,
            )

            py = pyp.tile([1, CHUNK], fp32)
            nc.tensor.matmul(py[:], ones_sb[:], masked[:], start=True, stop=True)
            y_sb = ypool.tile([1, CHUNK], fp32)
            nc.scalar.activation(y_sb[:], py[:], AF.Sigmoid)
            nc.sync.dma_start(y[0:1, n0:n0 + CHUNK], y_sb[:])

    nc.compile()
    return nc


# --------------------------------------------------------------------------
# v2: sorted tokens, per-block expert slots, 4 concurrent col-strips
# --------------------------------------------------------------------------
def _build_nc_v2(nsh):
    from contextlib import ExitStack

    import concourse.mybir as mybir
    import concourse.tile as tile

    fp32 = mybir.dt.float32
    fr = mybir.dt.float32r
    bf16 = mybir.dt.bfloat16
    AF = mybir.ActivationFunctionType
    OP = mybir.AluOpType

    nblk = nsh // CHUNK
    groups = nblk // STRIPS
    nc = _mk_bass(nsh)

    xT = nc.dram_tensor("xT", [D_IN, nsh], bf16, kind="ExternalInput")
    w1T = nc.dram_tensor("w1T", [D_IN, HID], bf16, kind="ExternalInput")
    b1c = nc.dram_tensor("b1c", [128, H_T], fp32, kind="ExternalInput")
    web = nc.dram_tensor("web", [HID, nblk * SLOTS_DMA], bf16, kind="ExternalInput")
    beh = nc.dram_tensor("beh", [128, groups], fp32, kind="ExternalInput")
    ohb = nc.dram_tensor("ohb", [128, groups * CHUNK], fp32, kind="ExternalInput")
    i4 = nc.dram_tensor("i4", [128, STRIPS], fr, kind="ExternalInput")
    y = nc.dram_tensor("y", [STRIPS, groups * CHUNK], fp32, kind="ExternalOutput")

    xT_v = xT.rearrange("(c p) n -> p c n", p=128)
    w1T_v = w1T.rearrange("(c p) h -> p c h", p=128)
    web_v = web.rearrange("(k p) q -> p k q", p=128)    # [128, H_T, nblk*SLOTS_DMA]

    with tile.TileContext(nc) as tc, ExitStack() as ctx:
        cpool = ctx.enter_context(tc.tile_pool(name="consts", bufs=1))
        xin = ctx.enter_context(tc.tile_pool(name="xin", bufs=3))
        ohin = ctx.enter_context(tc.tile_pool(name="ohin", bufs=3))
        hpool = ctx.enter_context(tc.tile_pool(name="h", bufs=5))
        mpool = ctx.enter_context(tc.tile_pool(name="masked", bufs=3))
        ypool = ctx.enter_context(tc.tile_pool(name="yrow", bufs=3))
        php = ctx.enter_context(tc.tile_pool(name="ph", bufs=4, space="PSUM"))
        psp = ctx.enter_context(tc.tile_pool(name="ps", bufs=2, space="PSUM"))
        pyp = ctx.enter_context(tc.tile_pool(name="py", bufs=2, space="PSUM"))

        w1_sb = cpool.tile([128, D_T, HID], bf16)
        web_dma = cpool.tile([128, H_T, nblk * SLOTS_DMA], bf16)
        web_sb = cpool.tile([128, H_T, nblk, SLOTS], bf16)
        b1_sb = cpool.tile([128, H_T], fp32)
        beh_sb = cpool.tile([128, groups], fp32)
        i4_sb = cpool.tile([128, STRIPS], fr)
        # w1/b1 gate the first matmul/relu: load them first on the sync queue;
        # bulk constants that are only needed later go via gpsimd queues.
        nc.sync.dma_start(w1_sb[:], w1T_v[:])
        nc.sync.dma_start(b1_sb[:], b1c[:])
        nc.gpsimd.dma_start(web_dma[:], web_v[:])
        nc.gpsimd.dma_start(beh_sb[:], beh[:])
        nc.gpsimd.dma_start(i4_sb[:], i4[:])
        # pad the compact slot table into 32-wide strips on-chip (unused
        # slot columns must be zero, not garbage, since their scores land
        # in PSUM rows that the mask op reads)
        nc.gpsimd.memset(web_sb[:], 0.0)
        nc.gpsimd.tensor_copy(
            web_sb[:, :, :, 0:SLOTS_DMA],
            web_dma.rearrange("p k (b s) -> p k b s", s=SLOTS_DMA),
        )

        for g in range(groups):

            def emit_chunk_a(j):
                ci = g * STRIPS + j
                n0 = ci * CHUNK
                xts = xin.tile([128, D_T, CHUNK], bf16, tag="xts")
                nc.sync.dma_start(xts[:], xT_v[:, :, n0:n0 + CHUNK])
                hT = hpool.tile([128, H_T, CHUNK], bf16, tag="hT")
                phs = []
                for k in range(H_T):
                    ph = php.tile([128, CHUNK], fp32, tag="ph")
                    for cc in range(D_T):
                        nc.tensor.matmul(
                            ph[:],
                            w1_sb[:, cc, 128 * k:128 * (k + 1)],
                            xts[:, cc, :],
                            start=(cc == 0),
                            stop=(cc == D_T - 1),
                        )
                    phs.append(ph)
                for k in range(H_T):
                    if k % 2 == 0:
                        nc.scalar.activation(
                            hT[:, k, :], phs[k][:], AF.Relu,
                            bias=b1_sb[:, k:k + 1], scale=1.0,
                        )
                    else:
                        nc.vector.tensor_scalar(
                            hT[:, k, :], phs[k][:],
                            b1_sb[:, k:k + 1], 0.0,
                            OP.add, OP.max,
                        )
                return hT

            def emit_b(ps, k, j):
                blk = g * STRIPS + j
                nc.tensor.matmul(
                    ps[32 * j:32 * (j + 1), :],
                    web_sb[:, k, blk, :],
                    hts[j][:, k, :],
                    start=(k == 0),
                    stop=(k == H_T - 1),
                    tile_position=(0, 32 * j),
                    skip_group_check=True,
                )

            hts = [emit_chunk_a(j) for j in range(STRIPS - 1)]

            ps = psp.tile([128, CHUNK], fp32)
            # stage B round k=0 for strips 0..2 rides inside the A phase so
            # the per-strip first-round weight loads hide under A matmuls
            for j in range(STRIPS - 1):
                emit_b(ps, 0, j)

            hts.append(emit_chunk_a(STRIPS - 1))

            oh_sb = ohin.tile([128, CHUNK], fp32)
            nc.sync.dma_start(oh_sb[:], ohb[:, g * CHUNK:(g + 1) * CHUNK])

            # remaining stage B: strip 3 k=0, then rounds k=1..7, 4 strips
            # concurrently (one per 32-col strip)
            emit_b(ps, 0, STRIPS - 1)
            for k in range(1, H_T):
                for j in range(STRIPS):
                    emit_b(ps, k, j)

            masked = mpool.tile([128, CHUNK], fr)
            nc.vector.scalar_tensor_tensor(
                masked[:], ps[:], beh_sb[:, g:g + 1], oh_sb[:],
                OP.add, OP.mult,
            )

            py = pyp.tile([STRIPS, CHUNK], fp32)
            nc.tensor.matmul(py[:], i4_sb[:], masked[:], start=True, stop=True)
            y_sb = ypool.tile([STRIPS, CHUNK], fp32)
            nc.scalar.activation(y_sb[:], py[:], AF.Sigmoid)
            nc.sync.dma_start(y[:, g * CHUNK:(g + 1) * CHUNK], y_sb[:])

    nc.compile()
    return nc


def _get_nc(version, nsh=NSH):
    key = (version, nsh)
    if key not in _BUILT:
        _BUILT[key] = (_build_nc_v2 if version == 2 else _build_nc_v1)(nsh)
    return _BUILT[key]


# --------------------------------------------------------------------------
# host prep
# --------------------------------------------------------------------------
def _common_prep(x, W1, b1, We, be, num, c):
    x = np.asarray(x, dtype=np.float32)
    W1 = np.asarray(W1, dtype=np.float32)
    b1 = np.asarray(b1, dtype=np.float32)
    We = np.asarray(We, dtype=np.float32)
    be = np.asarray(be, dtype=np.float32)
    eidx = np.asarray(np.asarray(c)[np.asarray(num)], dtype=np.int64)
    w1T = _tf32_round(W1.T)
    b1c = np.ascontiguousarray(b1.reshape(H_T, 128).T)
    return x, W1, b1, We, be, eidx, w1T, b1c


def _i4_mat():
    i4 = np.zeros((128, STRIPS), dtype=np.float32)
    for j in range(STRIPS):
        i4[32 * j:32 * (j + 1), j] = 1.0
    return i4


def _prep_core_v2(x_sh, e_sh, weT, be, w1T, b1c, i4, nsh):
    """One core's v2 input map. Returns (map, order) or None on overflow."""
    nblk = nsh // CHUNK
    groups = nblk // STRIPS
    order = np.argsort(e_sh, kind="stable")
    e_sorted = e_sh[order]

    web = np.zeros((HID, nblk * SLOTS_DMA), dtype=np.float32)
    beh = np.zeros((128, groups), dtype=np.float32)
    ohb = np.zeros((128, groups * CHUNK), dtype=np.float32)
    for b in range(nblk):
        be_blk = e_sorted[b * CHUNK:(b + 1) * CHUNK]
        experts, slot_of = np.unique(be_blk, return_inverse=True)
        if len(experts) > SLOTS_DMA:
            return None
        g, j = b // STRIPS, b % STRIPS
        web[:, b * SLOTS_DMA:b * SLOTS_DMA + len(experts)] = weT[:, experts]
        beh[32 * j:32 * j + len(experts), g] = be[experts, 0]
        ohb[32 * j + slot_of, g * CHUNK + np.arange(CHUNK)] = 1.0

    m = {
        "xT": np.ascontiguousarray(x_sh[order].T).astype(_bf16_dt()),
        "w1T": w1T.astype(_bf16_dt()),
        "b1c": b1c,
        "web": web.astype(_bf16_dt()),
        "beh": beh,
        "ohb": ohb,
        "i4": i4,
    }
    return m, order


def _unpermute_core_v2(yd, order, nsh):
    """Device output [STRIPS, groups*CHUNK] -> original token order [nsh]."""
    groups = (nsh // CHUNK) // STRIPS
    ys = np.ascontiguousarray(
        yd.reshape(STRIPS, groups, CHUNK).transpose(1, 0, 2)
    ).reshape(nsh)
    yc = np.empty(nsh, dtype=np.float32)
    yc[order] = ys
    return yc


def _host_prep_v2(x, W1, b1, We, be, num, c):
    """Per-core maps for v2 plus the per-core inverse permutations.

    Returns (in_maps, orders) or None if a block spans > SLOTS experts."""
    x, W1, b1, We, be, eidx, w1T, b1c = _common_prep(x, W1, b1, We, be, num, c)
    weT = We[:, 0, :].T                                  # [HID, E]
    i4 = _i4_mat()

    in_maps, orders = [], []
    for i in range(NCORES):
        sl = slice(i * NSH, (i + 1) * NSH)
        r = _prep_core_v2(x[sl], eidx[sl], weT, be, w1T, b1c, i4, NSH)
        if r is None:
            return None
        in_maps.append(r[0])
        orders.append(r[1])
    return in_maps, orders


def _host_prep_v1(x, W1, b1, We, be, num, c):
    x, W1, b1, We, be, eidx, w1T, b1c = _common_prep(x, W1, b1, We, be, num, c)
    weT = np.zeros((HID, EP), dtype=np.float32)
    weT[:, :E] = We[:, 0, :].T
    weT = _tf32_round(weT)
    bec = np.zeros((EP, 1), dtype=np.float32)
    bec[:E, 0] = be[:, 0]
    oh_full = np.zeros((EP, N), dtype=np.float32)
    oh_full[eidx, np.arange(N)] = 1.0

    in_maps = []
    for i in range(NCORES):
        sl = slice(i * NSH, (i + 1) * NSH)
        in_maps.append({
            "xT": _tf32_round(x[sl].T),
            "w1T": w1T,
            "b1c": b1c,
            "weT": weT,
            "bec": bec,
            "oh": np.ascontiguousarray(oh_full[:, sl]),
        })
    return in_maps


def kernel(x, W1, b1, We, be, num, c):
    global LAST_RESULTS
    from concourse.bass_utils import run_bass_kernel_spmd

    prep = _host_prep_v2(x, W1, b1, We, be, num, c)
    if prep is not None:
        in_maps, orders = prep
        nc = _get_nc(2, NSH)
        res = run_bass_kernel_spmd(
            nc, in_maps, core_ids=list(range(NCORES)), trace=TRACE,
        )
        LAST_RESULTS = res
        out = np.empty(N, dtype=np.float32)
        for i in range(NCORES):
            out[i * NSH:(i + 1) * NSH] = _unpermute_core_v2(
                res.results[i]["y"], orders[i], NSH)
        return out.reshape(N, 1)

    in_maps = _host_prep_v1(x, W1, b1, We, be, num, c)
    nc = _get_nc(1, NSH)
    res = run_bass_kernel_spmd(
        nc, in_maps, core_ids=list(range(NCORES)), trace=TRACE,
    )
    LAST_RESULTS = res
    out = np.concatenate([r["y"].reshape(NSH) for r in res.results])
    return out.reshape(N, 1).astype(np.float32)
